# revision 33
# baseline (speedup 1.0000x reference)
"""CPM loss on 8 Trainium2 NeuronCores via Bass/Tile.

Strategy (data-parallel over B, 64 samples per core, no collectives):
  - host: all index bookkeeping from pids/camids; inputs cast to bf16
    (measured end-to-end loss error ~1e-4 relative, far below fp32 noise
    amplified by the hinge).
  - device, per core:
      cross centers: cross = S^T (A^T F) = (A S)^T F, with M = A*S (with
        1/count folded in) precomputed on host -> one chain of 48 bf16
        matmuls over the FULL f_original (bf16, 6 MB) -> per-sample cross
        centers in PSUM -> one copy to SBUF. Cheaper and far faster than
        the ~110us AllReduce of per-core partial center sums.
      main:  per k-tile [128,3072]: push diff (g - f_orig) on GPSIMD+DVE,
             pull diff (g - cross) on DVE, Square+accumulate over D
             (ACT x4, DVE tensor_tensor_reduce x2) -> d^2 per
             (sample-half, k, part).
      tail:  sqrt, hinge = relu((d_pull + margin - d_push) * w) accumulated
             per partition -> [128,1] partial out.
  - host: sum the 8 partial outputs, divide by 6 * id_count.

Partition layout: q = 2*local_b + h, h = p//3; free dim = (pp = p%3, d).
"""
import re as _re
import sys

import numpy as np

if "/opt/trn_rl_repo" not in sys.path:
    sys.path.insert(0, "/opt/trn_rl_repo")

import bass_rust
import ml_dtypes
import concourse.bass as bass
import concourse.mybir as mybir
import concourse.bass_utils as bass_utils
from concourse import tile
from concourse.vector_clock import ScopedClock

F32 = mybir.dt.float32
BF16 = mybir.dt.bfloat16
AFT = mybir.ActivationFunctionType
BF = ml_dtypes.bfloat16

MARGIN = 0.2
B, K, P, D = 512, 8, 6, 1024
NID = 64
NCORES = 8
BC = B // NCORES          # 64 local samples per core
HFREE = 3 * D             # 3072 free elements per partition row
NQ = 2 * BC               # 128 partition rows
NT = B // 128             # 4 global sample chunks for the centers matmul

# The walrus build in this image rejects instructions carrying more than one
# semaphore wait ("Too many sync wait commands"). Two mitigations:
#  1. TileContext's exit drain normally carries every outstanding wait ->
#     patched to spread waits over a chain of SP nops.
#  2. A post-pass splits any remaining multi-wait instruction by hoisting
#     excess waits onto same-engine nops inserted right before it.
_MAX_WAITS = 1


def _patched_drain_and_barrier(self, tick_clock, wait_clock):
    gc = tick_clock.global_clock
    vals = [int(s) for s in _re.findall(r"-?\d+", repr(gc))]
    procs = [p for p, v in enumerate(vals) if v > 0]
    for i in range(0, len(procs), _MAX_WAITS):
        sub = bass_rust.VectorClock()
        for p in procs[i : i + _MAX_WAITS]:
            sub.require_at_least(p, vals[p])
        nop = self.nc.sync.nop(nofuse=True, hint="drain_wait_split")
        wait_clock.add_sem_waits(nop.ins, ScopedClock({None: sub}))
    self.nc.sync.drain()
    self.nc.all_engine_barrier()
    assert self.sems is not None
    popped = self.nc._tile_sem_poison_stack.pop()
    assert popped is self._sem_poison
    self.nc.clear_and_free_semaphores(list(self.sems.allocated().values()))
    self.nc.all_engine_barrier()


tile.TileContext._drain_and_barrier = _patched_drain_and_barrier


def _split_excess_waits(nc, max_waits=_MAX_WAITS):
    """Hoist excess per-instruction sem waits onto same-engine NoOps."""
    n_split = 0
    for bb in nc.main_func.blocks:
        insts = bb.instructions
        out = []
        for ins in insts:
            si = ins.sync_info
            waits = list(si.on_wait) if si is not None and si.on_wait else []
            if len(waits) > max_waits:
                extra, keep = waits[:-max_waits], waits[-max_waits:]
                for j in range(0, len(extra), max_waits):
                    nop = mybir.InstNoOp(
                        name=f"waitsplit-{n_split}-{j}", ins=[], outs=[]
                    )
                    nop.engine = ins.engine
                    nop.sync_info = mybir.SyncInfo(
                        on_wait=extra[j : j + max_waits], on_update=[]
                    )
                    out.append(nop)
                ins.sync_info = mybir.SyncInfo(
                    on_wait=keep, on_update=list(si.on_update or [])
                )
                n_split += 1
            out.append(ins)
        if len(out) != len(insts):
            bb.instructions = out
    return n_split


_NC_CACHE = None
import os
USE_TTR = os.environ.get("USE_TTR", "1") == "1"


def _build_nc(ntc):
    global _NC_CACHE
    if _NC_CACHE is not None and _NC_CACHE[0] == ntc:
        return _NC_CACHE[1]
    NT = ntc
    nc = bass.Bass()

    fgen = nc.dram_tensor("fgen", [K, NQ, HFREE], BF16, kind="ExternalInput")
    forig = nc.dram_tensor("forig", [NQ, HFREE], BF16, kind="ExternalInput")
    fofull = nc.dram_tensor("fofull", [NT, 128, 2 * HFREE], BF16, kind="ExternalInput")
    mmat = nc.dram_tensor("mmat", [NT, 2, 128, NQ], BF16, kind="ExternalInput")
    wvec = nc.dram_tensor("wvec", [NQ, 1], F32, kind="ExternalInput")
    out = nc.dram_tensor("out", [NQ, 1], F32, kind="ExternalOutput")

    NSEG = 6                # 512-wide matmul chunks per 3072
    SEG = HFREE // NSEG

    with tile.TileContext(nc) as tc:
        with (
            tc.tile_pool(name="const", bufs=1) as cpool,
            tc.tile_pool(name="gpool", bufs=1) as gpool,
            tc.tile_pool(name="dpush", bufs=5) as dpush_pool,
            tc.tile_pool(name="scr", bufs=4) as scr_pool,
            tc.tile_pool(name="psumX", bufs=1, space="PSUM") as psumX,
        ):
            # ---- loads. Small constants via HWDGE (sync). Bulk tiles via
            # SWDGE (gpsimd.dma_start): one SWDGE transfer fans out across
            # all 16 SDMA engines, so tiles arrive in issue order at full
            # aggregate bandwidth instead of trickling at one queue's ~30
            # GB/s. GPSIMD runs no compute in this kernel, so SWDGE
            # descriptor generation has the engine to itself. Order: fo_q
            # and g0 first (gates push k=0), then fof chunks (gate the
            # cross-center matmuls), then the remaining g tiles. ----
            fo_q = cpool.tile([NQ, HFREE], BF16)
            gt = [gpool.tile([NQ, HFREE], BF16, name=f"g{k}", tag=f"g{k}") for k in range(K)]
            fof = [cpool.tile([128, 2 * HFREE], BF16, name=f"fof{t}", tag=f"fof{t}") for t in range(NT)]

            # Two concurrent DMA silos. Each HWDGE queue moves only ~27
            # GB/s, so the latency-critical tiles are striped WIDE across
            # queues (8 partition-slices each for fo_q/g0, 4 for the fof
            # center inputs); the remaining g tiles stream via SWDGE
            # (gpsimd), a separate silo with its own ~160 GB/s aggregate.
            def hload(dst, src, nstripe):
                step = 128 // nstripe
                for s in range(nstripe):
                    sl = slice(s * step, (s + 1) * step)
                    nc.sync.dma_start(dst[sl, :], src[sl, :])

            hload(fo_q, forig, 8)
            hload(gt[0], fgen[0], 8)
            mm = [
                [cpool.tile([128, NQ], BF16, name=f"mm{t}{h}", tag=f"mm{t}{h}") for h in (0, 1)]
                for t in range(NT)
            ]
            for t in range(NT):
                for h in (0, 1):
                    nc.sync.dma_start(mm[t][h][:], mmat[t, h])
            for t in range(NT):
                hload(fof[t], fofull[t], 4)
            wv = cpool.tile([NQ, 1], F32, tag="wv")
            nc.sync.dma_start(wv[:], wvec[:])
            for k in range(1, K):
                nc.gpsimd.dma_start(gt[k][:], fgen[k])

            dsq_push = cpool.tile([NQ, 3 * K], F32, tag="dsq_push")
            dsq_pull = cpool.tile([NQ, 3 * K], F32, tag="dsq_pull")

            # Square-accumulates: ACT is the cheap reducer (short
            # accumulator-read); DVE takes ~12 of the 48 segments as fused
            # mult+reduce STTs to balance the two engines.
            def sqacc_act(diff, pp, dst):
                scr = scr_pool.tile([NQ, D], BF16, tag="ascr")
                nc.scalar.activation(
                    scr[:], diff[:, pp * D : (pp + 1) * D], AFT.Square,
                    accum_out=dst,
                )

            def sqacc_dve(diff, pp, dst):
                seg = slice(pp * D, (pp + 1) * D)
                scr = scr_pool.tile([NQ, D], BF16, tag="vscr")
                nc.vector.scalar_tensor_tensor(
                    scr[:], diff[:, seg], 1.0, diff[:, seg],
                    op0=mybir.AluOpType.mult, op1=mybir.AluOpType.mult,
                    accum_out=dst,
                )

            # push[k] depends only on (g_k, fo_q) -> hideable under DMA.
            def emit_push(k):
                dpu = dpush_pool.tile([NQ, HFREE], BF16, name=f"dpu{k}", tag="dpu")
                nc.vector.tensor_sub(dpu[:], gt[k][:], fo_q[:])
                col0 = 3 * k
                for pp in range(3):
                    dst = dsq_push[:, col0 + pp : col0 + pp + 1]
                    if pp == 2 and k % 2 == 0:
                        sqacc_dve(dpu, pp, dst)
                    else:
                        sqacc_act(dpu, pp, dst)

            def emit_pull(k):
                dpl = dpush_pool.tile([NQ, HFREE], BF16, name=f"dpl{k}", tag="dpl")
                nc.vector.tensor_sub(dpl[:], gt[k][:], cross[:])
                col0 = 3 * k
                for pp in range(3):
                    dst = dsq_pull[:, col0 + pp : col0 + pp + 1]
                    if pp == 2:
                        sqacc_dve(dpl, pp, dst)
                    else:
                        sqacc_act(dpl, pp, dst)

            for k in range(2):
                emit_push(k)

            # ---- cross centers: (A S)^T @ F, 48 bf16 matmuls, chunk-outer
            # so the PE chain pipelines with the fof chunk arrivals ----
            xps = psumX.tile([NQ, HFREE], F32)
            cross = cpool.tile([NQ, HFREE], BF16, tag="cross")
            for ti, (t, h) in enumerate(
                [(t, h) for t in range(NT) for h in (0, 1)]
            ):
                for j in range(NSEG):
                    seg = slice(j * SEG, (j + 1) * SEG)
                    rhs = fof[t][:, h * HFREE + j * SEG : h * HFREE + (j + 1) * SEG]
                    nc.tensor.matmul(
                        xps[:, seg], mm[t][h][:], rhs,
                        start=(ti == 0), stop=(ti == 2 * NT - 1),
                    )
            nc.scalar.copy(cross[:], xps[:])

            for k in range(2, K):
                emit_push(k)
                emit_pull(k - 2)
            emit_pull(K - 2)
            emit_pull(K - 1)

            # ---- tail ----
            d_push = cpool.tile([NQ, 3 * K], F32, tag="d_push")
            d_pull = cpool.tile([NQ, 3 * K], F32, tag="d_pull")
            nc.scalar.activation(d_push[:], dsq_push[:], AFT.Sqrt)
            nc.scalar.activation(d_pull[:], dsq_pull[:], AFT.Sqrt)
            targ = cpool.tile([NQ, 3 * K], F32, tag="targ")
            # (d_pull + margin) - d_push
            nc.vector.scalar_tensor_tensor(
                targ[:], d_pull[:], MARGIN, d_push[:],
                op0=mybir.AluOpType.add, op1=mybir.AluOpType.subtract,
            )
            relu_scr = cpool.tile([NQ, 3 * K], F32, tag="relu_scr")
            acc = cpool.tile([NQ, 1], F32, tag="acc")
            # relu(w_q * targ) summed over (k, pp); w_q >= 0 so this
            # equals w_q * relu(targ) summed.
            nc.scalar.activation(
                relu_scr[:], targ[:], AFT.Relu, scale=wv[:], accum_out=acc[:]
            )
            nc.sync.dma_start(out[:], acc[:])

    mybir.codegen_inst_isa_subclasses(nc)   # extended-ISA instr bytes (TTR)
    _split_excess_waits(nc)
    _NC_CACHE = (ntc, nc)
    return nc


def _host_prep(f_original, f_generated, pids, camids):
    """Host bookkeeping + per-core input maps."""
    f_original = np.asarray(f_original, dtype=np.float32)
    f_generated = np.asarray(f_generated, dtype=np.float32)
    pids = np.asarray(pids).astype(np.int64)
    camids = np.asarray(camids).astype(np.int64)

    mod = (camids != 0).astype(np.int64)          # 0 = rgb, 1 = sar
    cnt = np.zeros((2, NID), dtype=np.float32)
    np.add.at(cnt, (mod, pids), 1.0)
    valid_id = (cnt[0] > 0) & (cnt[1] > 0)
    id_count = float(valid_id.sum())
    denom = max(id_count, 1.0)

    own_row = (pids + NID * mod).astype(np.int64)          # [B]
    cross_row = (pids + NID * (1 - mod)).astype(np.int64)  # [B]
    inv_cnt2 = (1.0 / np.maximum(cnt.reshape(-1), 1.0)).astype(np.float32)
    grp_cnt = cnt[mod, pids]
    w = np.where(valid_id[pids], 1.0 / (np.maximum(grp_cnt, 1.0) * K), 0.0)
    w = w.astype(np.float32)

    fo_bf = f_original.astype(BF).reshape(B, 2 * HFREE)   # [B, 6144]
    fg_bf = f_generated.astype(BF)

    # Per core, only the f_original rows whose (pid, modality) feeds one of
    # the core's cross centers are needed for the center matmul ("fofc").
    contribs = []
    for c in range(NCORES):
        sl = slice(c * BC, (c + 1) * BC)
        rows_needed = np.unique(cross_row[sl])
        contribs.append(np.nonzero(np.isin(own_row, rows_needed))[0])
    ntc = max(1, max((len(cb) + 127) // 128 for cb in contribs))

    in_maps = []
    for c in range(NCORES):
        sl = slice(c * BC, (c + 1) * BC)
        fg = (
            fg_bf[sl]
            .transpose(1, 0, 2, 3)
            .reshape(K, BC, 2, HFREE)
            .reshape(K, NQ, HFREE)
        )
        fo = fo_bf[sl].reshape(NQ, HFREE)
        contrib = contribs[c]
        npad = ntc * 128
        cpad = np.zeros(npad, dtype=np.int64)
        cpad[: len(contrib)] = contrib
        fofc = fo_bf[cpad].reshape(ntc, 128, 2 * HFREE)
        pos_of = {int(gi): pos for pos, gi in enumerate(contrib)}
        # mmat[t, h, i, q] = inv_cnt[cross_row[b]] where packed sample
        # 128t+i has own_row == cross_row[b(q)] and q = 2b+h.
        mm = np.zeros((ntc, 2, 128, NQ), dtype=np.float32)
        for bl in range(BC):
            b = c * BC + bl
            r = cross_row[b]
            s = inv_cnt2[r]
            for gi in np.nonzero(own_row == r)[0]:
                t, i = divmod(pos_of[int(gi)], 128)
                mm[t, 0, i, 2 * bl] = s
                mm[t, 1, i, 2 * bl + 1] = s
        wq = np.zeros((NQ, 1), dtype=np.float32)
        wq[0::2, 0] = w[sl]
        wq[1::2, 0] = w[sl]
        in_maps.append(
            {
                "fgen": np.ascontiguousarray(fg),
                "forig": np.ascontiguousarray(fo),
                "fofull": np.ascontiguousarray(fofc),
                "mmat": mm.astype(BF),
                "wvec": wq,
            }
        )
    return in_maps, id_count, denom, ntc


def run_device(f_original, f_generated, pids, camids, **spmd_kwargs):
    """Build + run; returns (loss, BassKernelResults)."""
    in_maps, id_count, denom, ntc = _host_prep(f_original, f_generated, pids, camids)
    nc = _build_nc(ntc)
    res = bass_utils.run_bass_kernel_spmd(
        nc, in_maps, core_ids=list(range(NCORES)), **spmd_kwargs
    )
    total = float(sum(r["out"].sum() for r in res.results))
    loss = np.float32(total / (P * denom)) if id_count > 0 else np.float32(0.0)
    return np.asarray(loss, dtype=np.float32), res


def kernel(f_original, f_generated, pids, camids):
    loss, _ = run_device(f_original, f_generated, pids, camids)
    return loss


# revision 35
# speedup vs baseline: 1.0566x; 1.0566x over previous
"""CPM loss on 8 Trainium2 NeuronCores via Bass/Tile.

Strategy (data-parallel over B, 64 samples per core, no collectives):
  - host: all index bookkeeping from pids/camids; inputs cast to bf16
    (measured end-to-end loss error ~1e-4 relative, far below fp32 noise
    amplified by the hinge).
  - device, per core:
      cross centers: cross = S^T (A^T F) = (A S)^T F, with M = A*S (with
        1/count folded in) precomputed on host -> one chain of 48 bf16
        matmuls over the FULL f_original (bf16, 6 MB) -> per-sample cross
        centers in PSUM -> one copy to SBUF. Cheaper and far faster than
        the ~110us AllReduce of per-core partial center sums.
      main:  per k-tile [128,3072]: push diff (g - f_orig) on GPSIMD+DVE,
             pull diff (g - cross) on DVE, Square+accumulate over D
             (ACT x4, DVE tensor_tensor_reduce x2) -> d^2 per
             (sample-half, k, part).
      tail:  sqrt, hinge = relu((d_pull + margin - d_push) * w) accumulated
             per partition -> [128,1] partial out.
  - host: sum the 8 partial outputs, divide by 6 * id_count.

Partition layout: q = 2*local_b + h, h = p//3; free dim = (pp = p%3, d).
"""
import re as _re
import sys

import numpy as np

if "/opt/trn_rl_repo" not in sys.path:
    sys.path.insert(0, "/opt/trn_rl_repo")

import bass_rust
import ml_dtypes
import concourse.bass as bass
import concourse.mybir as mybir
import concourse.bass_utils as bass_utils
from concourse import tile
from concourse.vector_clock import ScopedClock

F32 = mybir.dt.float32
BF16 = mybir.dt.bfloat16
AFT = mybir.ActivationFunctionType
BF = ml_dtypes.bfloat16

MARGIN = 0.2
B, K, P, D = 512, 8, 6, 1024
NID = 64
NCORES = 8
BC = B // NCORES          # 64 local samples per core
HFREE = 3 * D             # 3072 free elements per partition row
NQ = 2 * BC               # 128 partition rows
NT = B // 128             # 4 global sample chunks for the centers matmul

# The walrus build in this image rejects instructions carrying more than one
# semaphore wait ("Too many sync wait commands"). Two mitigations:
#  1. TileContext's exit drain normally carries every outstanding wait ->
#     patched to spread waits over a chain of SP nops.
#  2. A post-pass splits any remaining multi-wait instruction by hoisting
#     excess waits onto same-engine nops inserted right before it.
_MAX_WAITS = 1


def _patched_drain_and_barrier(self, tick_clock, wait_clock):
    gc = tick_clock.global_clock
    vals = [int(s) for s in _re.findall(r"-?\d+", repr(gc))]
    procs = [p for p, v in enumerate(vals) if v > 0]
    for i in range(0, len(procs), _MAX_WAITS):
        sub = bass_rust.VectorClock()
        for p in procs[i : i + _MAX_WAITS]:
            sub.require_at_least(p, vals[p])
        nop = self.nc.sync.nop(nofuse=True, hint="drain_wait_split")
        wait_clock.add_sem_waits(nop.ins, ScopedClock({None: sub}))
    self.nc.sync.drain()
    self.nc.all_engine_barrier()
    assert self.sems is not None
    popped = self.nc._tile_sem_poison_stack.pop()
    assert popped is self._sem_poison
    self.nc.clear_and_free_semaphores(list(self.sems.allocated().values()))
    self.nc.all_engine_barrier()


tile.TileContext._drain_and_barrier = _patched_drain_and_barrier


def _split_excess_waits(nc, max_waits=_MAX_WAITS):
    """Hoist excess per-instruction sem waits onto same-engine NoOps."""
    n_split = 0
    for bb in nc.main_func.blocks:
        insts = bb.instructions
        out = []
        for ins in insts:
            si = ins.sync_info
            waits = list(si.on_wait) if si is not None and si.on_wait else []
            if len(waits) > max_waits:
                extra, keep = waits[:-max_waits], waits[-max_waits:]
                for j in range(0, len(extra), max_waits):
                    nop = mybir.InstNoOp(
                        name=f"waitsplit-{n_split}-{j}", ins=[], outs=[]
                    )
                    nop.engine = ins.engine
                    nop.sync_info = mybir.SyncInfo(
                        on_wait=extra[j : j + max_waits], on_update=[]
                    )
                    out.append(nop)
                ins.sync_info = mybir.SyncInfo(
                    on_wait=keep, on_update=list(si.on_update or [])
                )
                n_split += 1
            out.append(ins)
        if len(out) != len(insts):
            bb.instructions = out
    return n_split


_NC_CACHE = None
import os
USE_TTR = os.environ.get("USE_TTR", "1") == "1"


def _build_nc(ntc):
    global _NC_CACHE
    if _NC_CACHE is not None and _NC_CACHE[0] == ntc:
        return _NC_CACHE[1]
    NT = ntc
    nc = bass.Bass()

    fgen = nc.dram_tensor("fgen", [K, NQ, HFREE], BF16, kind="ExternalInput")
    forig = nc.dram_tensor("forig", [NQ, HFREE], BF16, kind="ExternalInput")
    fofull = nc.dram_tensor("fofull", [NT, 128, 2 * HFREE], BF16, kind="ExternalInput")
    mmat = nc.dram_tensor("mmat", [NT, 2, 128, NQ], BF16, kind="ExternalInput")
    wvec = nc.dram_tensor("wvec", [NQ, 1], F32, kind="ExternalInput")
    out = nc.dram_tensor("out", [NQ, 1], F32, kind="ExternalOutput")

    NSEG = 6                # 512-wide matmul chunks per 3072
    SEG = HFREE // NSEG

    with tile.TileContext(nc) as tc:
        with (
            tc.tile_pool(name="const", bufs=1) as cpool,
            tc.tile_pool(name="gpool", bufs=1) as gpool,
            tc.tile_pool(name="dpush", bufs=5) as dpush_pool,
            tc.tile_pool(name="scr", bufs=4) as scr_pool,
            tc.tile_pool(name="psumX", bufs=1, space="PSUM") as psumX,
        ):
            # ---- loads. Small constants via HWDGE (sync). Bulk tiles via
            # SWDGE (gpsimd.dma_start): one SWDGE transfer fans out across
            # all 16 SDMA engines, so tiles arrive in issue order at full
            # aggregate bandwidth instead of trickling at one queue's ~30
            # GB/s. GPSIMD runs no compute in this kernel, so SWDGE
            # descriptor generation has the engine to itself. Order: fo_q
            # and g0 first (gates push k=0), then fof chunks (gate the
            # cross-center matmuls), then the remaining g tiles. ----
            fo_q = cpool.tile([NQ, HFREE], BF16)
            gt = [gpool.tile([NQ, HFREE], BF16, name=f"g{k}", tag=f"g{k}") for k in range(K)]
            fof = [cpool.tile([128, 2 * HFREE], BF16, name=f"fof{t}", tag=f"fof{t}") for t in range(NT)]

            # Two concurrent DMA silos. Each HWDGE queue moves only ~27
            # GB/s, so the latency-critical tiles are striped WIDE across
            # queues (8 partition-slices each for fo_q/g0, 4 for the fof
            # center inputs); the remaining g tiles stream via SWDGE
            # (gpsimd), a separate silo with its own ~160 GB/s aggregate.
            def hload(dst, src, nstripe):
                step = 128 // nstripe
                for s in range(nstripe):
                    sl = slice(s * step, (s + 1) * step)
                    nc.sync.dma_start(dst[sl, :], src[sl, :])

            hload(fo_q, forig, 8)
            hload(gt[0], fgen[0], 8)
            mm = [
                [cpool.tile([128, NQ], BF16, name=f"mm{t}{h}", tag=f"mm{t}{h}") for h in (0, 1)]
                for t in range(NT)
            ]
            for t in range(NT):
                for h in (0, 1):
                    nc.sync.dma_start(mm[t][h][:], mmat[t, h])
            for t in range(NT):
                hload(fof[t], fofull[t], 4)
            wv = cpool.tile([NQ, 1], F32, tag="wv")
            nc.sync.dma_start(wv[:], wvec[:])
            # g1..g4 via the SWDGE silo (done before any DVE
            # tensor-scalar-class op can lock the shared SBUF port and
            # starve its descriptor generation); g5..g7 striped on HWDGE.
            for k in range(1, 5):
                nc.gpsimd.dma_start(gt[k][:], fgen[k])
            for k in range(5, K):
                hload(gt[k], fgen[k], 4)

            dsq_push = cpool.tile([NQ, 3 * K], F32, tag="dsq_push")
            dsq_pull = cpool.tile([NQ, 3 * K], F32, tag="dsq_pull")

            # Square-accumulates: ACT is the cheap reducer (short
            # accumulator-read); DVE takes ~12 of the 48 segments as fused
            # mult+reduce STTs to balance the two engines.
            def sqacc_act(diff, pp, dst):
                scr = scr_pool.tile([NQ, D], BF16, tag="ascr")
                nc.scalar.activation(
                    scr[:], diff[:, pp * D : (pp + 1) * D], AFT.Square,
                    accum_out=dst,
                )

            def sqacc_dve(diff, pp, dst):
                seg = slice(pp * D, (pp + 1) * D)
                scr = scr_pool.tile([NQ, D], BF16, tag="vscr")
                nc.vector.scalar_tensor_tensor(
                    scr[:], diff[:, seg], 1.0, diff[:, seg],
                    op0=mybir.AluOpType.mult, op1=mybir.AluOpType.mult,
                    accum_out=dst,
                )

            # push[k] depends only on (g_k, fo_q) -> hideable under DMA.
            # All push square-accumulates go to ACT: a DVE STT here could
            # grab the shared SBUF port and starve SWDGE descriptor
            # generation mid-DMA.
            def emit_push(k):
                dpu = dpush_pool.tile([NQ, HFREE], BF16, name=f"dpu{k}", tag="dpu")
                nc.vector.tensor_sub(dpu[:], gt[k][:], fo_q[:])
                col0 = 3 * k
                for pp in range(3):
                    sqacc_act(dpu, pp, dsq_push[:, col0 + pp : col0 + pp + 1])

            def emit_pull(k):
                dpl = dpush_pool.tile([NQ, HFREE], BF16, name=f"dpl{k}", tag="dpl")
                nc.vector.tensor_sub(dpl[:], gt[k][:], cross[:])
                col0 = 3 * k
                for pp in range(3):
                    dst = dsq_pull[:, col0 + pp : col0 + pp + 1]
                    if pp == 2 or (pp == 1 and k % 2 == 0):
                        sqacc_dve(dpl, pp, dst)
                    else:
                        sqacc_act(dpl, pp, dst)

            for k in range(2):
                emit_push(k)

            # ---- cross centers: (A S)^T @ F, 48 bf16 matmuls, chunk-outer
            # so the PE chain pipelines with the fof chunk arrivals ----
            xps = psumX.tile([NQ, HFREE], F32)
            cross = cpool.tile([NQ, HFREE], BF16, tag="cross")
            for ti, (t, h) in enumerate(
                [(t, h) for t in range(NT) for h in (0, 1)]
            ):
                for j in range(NSEG):
                    seg = slice(j * SEG, (j + 1) * SEG)
                    rhs = fof[t][:, h * HFREE + j * SEG : h * HFREE + (j + 1) * SEG]
                    nc.tensor.matmul(
                        xps[:, seg], mm[t][h][:], rhs,
                        start=(ti == 0), stop=(ti == 2 * NT - 1),
                    )
            nc.scalar.copy(cross[:], xps[:])

            for k in range(2, K):
                emit_push(k)
                emit_pull(k - 2)
            emit_pull(K - 2)
            emit_pull(K - 1)

            # ---- tail ----
            d_push = cpool.tile([NQ, 3 * K], F32, tag="d_push")
            d_pull = cpool.tile([NQ, 3 * K], F32, tag="d_pull")
            nc.scalar.activation(d_push[:], dsq_push[:], AFT.Sqrt)
            nc.scalar.activation(d_pull[:], dsq_pull[:], AFT.Sqrt)
            targ = cpool.tile([NQ, 3 * K], F32, tag="targ")
            # (d_pull + margin) - d_push
            nc.vector.scalar_tensor_tensor(
                targ[:], d_pull[:], MARGIN, d_push[:],
                op0=mybir.AluOpType.add, op1=mybir.AluOpType.subtract,
            )
            relu_scr = cpool.tile([NQ, 3 * K], F32, tag="relu_scr")
            acc = cpool.tile([NQ, 1], F32, tag="acc")
            # relu(w_q * targ) summed over (k, pp); w_q >= 0 so this
            # equals w_q * relu(targ) summed.
            nc.scalar.activation(
                relu_scr[:], targ[:], AFT.Relu, scale=wv[:], accum_out=acc[:]
            )
            nc.sync.dma_start(out[:], acc[:])

    mybir.codegen_inst_isa_subclasses(nc)   # extended-ISA instr bytes (TTR)
    _split_excess_waits(nc)
    _NC_CACHE = (ntc, nc)
    return nc


def _host_prep(f_original, f_generated, pids, camids):
    """Host bookkeeping + per-core input maps."""
    f_original = np.asarray(f_original, dtype=np.float32)
    f_generated = np.asarray(f_generated, dtype=np.float32)
    pids = np.asarray(pids).astype(np.int64)
    camids = np.asarray(camids).astype(np.int64)

    mod = (camids != 0).astype(np.int64)          # 0 = rgb, 1 = sar
    cnt = np.zeros((2, NID), dtype=np.float32)
    np.add.at(cnt, (mod, pids), 1.0)
    valid_id = (cnt[0] > 0) & (cnt[1] > 0)
    id_count = float(valid_id.sum())
    denom = max(id_count, 1.0)

    own_row = (pids + NID * mod).astype(np.int64)          # [B]
    cross_row = (pids + NID * (1 - mod)).astype(np.int64)  # [B]
    inv_cnt2 = (1.0 / np.maximum(cnt.reshape(-1), 1.0)).astype(np.float32)
    grp_cnt = cnt[mod, pids]
    w = np.where(valid_id[pids], 1.0 / (np.maximum(grp_cnt, 1.0) * K), 0.0)
    w = w.astype(np.float32)

    fo_bf = f_original.astype(BF).reshape(B, 2 * HFREE)   # [B, 6144]
    fg_bf = f_generated.astype(BF)

    # Per core, only the f_original rows whose (pid, modality) feeds one of
    # the core's cross centers are needed for the center matmul ("fofc").
    contribs = []
    for c in range(NCORES):
        sl = slice(c * BC, (c + 1) * BC)
        rows_needed = np.unique(cross_row[sl])
        contribs.append(np.nonzero(np.isin(own_row, rows_needed))[0])
    ntc = max(1, max((len(cb) + 127) // 128 for cb in contribs))

    in_maps = []
    for c in range(NCORES):
        sl = slice(c * BC, (c + 1) * BC)
        fg = (
            fg_bf[sl]
            .transpose(1, 0, 2, 3)
            .reshape(K, BC, 2, HFREE)
            .reshape(K, NQ, HFREE)
        )
        fo = fo_bf[sl].reshape(NQ, HFREE)
        contrib = contribs[c]
        npad = ntc * 128
        cpad = np.zeros(npad, dtype=np.int64)
        cpad[: len(contrib)] = contrib
        fofc = fo_bf[cpad].reshape(ntc, 128, 2 * HFREE)
        pos_of = {int(gi): pos for pos, gi in enumerate(contrib)}
        # mmat[t, h, i, q] = inv_cnt[cross_row[b]] where packed sample
        # 128t+i has own_row == cross_row[b(q)] and q = 2b+h.
        mm = np.zeros((ntc, 2, 128, NQ), dtype=np.float32)
        for bl in range(BC):
            b = c * BC + bl
            r = cross_row[b]
            s = inv_cnt2[r]
            for gi in np.nonzero(own_row == r)[0]:
                t, i = divmod(pos_of[int(gi)], 128)
                mm[t, 0, i, 2 * bl] = s
                mm[t, 1, i, 2 * bl + 1] = s
        wq = np.zeros((NQ, 1), dtype=np.float32)
        wq[0::2, 0] = w[sl]
        wq[1::2, 0] = w[sl]
        in_maps.append(
            {
                "fgen": np.ascontiguousarray(fg),
                "forig": np.ascontiguousarray(fo),
                "fofull": np.ascontiguousarray(fofc),
                "mmat": mm.astype(BF),
                "wvec": wq,
            }
        )
    return in_maps, id_count, denom, ntc


def run_device(f_original, f_generated, pids, camids, **spmd_kwargs):
    """Build + run; returns (loss, BassKernelResults)."""
    in_maps, id_count, denom, ntc = _host_prep(f_original, f_generated, pids, camids)
    nc = _build_nc(ntc)
    res = bass_utils.run_bass_kernel_spmd(
        nc, in_maps, core_ids=list(range(NCORES)), **spmd_kwargs
    )
    total = float(sum(r["out"].sum() for r in res.results))
    loss = np.float32(total / (P * denom)) if id_count > 0 else np.float32(0.0)
    return np.asarray(loss, dtype=np.float32), res


def kernel(f_original, f_generated, pids, camids):
    loss, _ = run_device(f_original, f_generated, pids, camids)
    return loss


# revision 37
# speedup vs baseline: 1.4956x; 1.4155x over previous
"""CPM loss on 8 Trainium2 NeuronCores via Bass/Tile.

Strategy (data-parallel over B, 64 samples per core, no collectives):
  - host: all index bookkeeping from pids/camids; inputs cast to bf16
    (measured end-to-end loss error ~1e-4 relative, far below fp32 noise
    amplified by the hinge).
  - device, per core:
      cross centers: cross = S^T (A^T F) = (A S)^T F, with M = A*S (with
        1/count folded in) precomputed on host -> one chain of 48 bf16
        matmuls over the FULL f_original (bf16, 6 MB) -> per-sample cross
        centers in PSUM -> one copy to SBUF. Cheaper and far faster than
        the ~110us AllReduce of per-core partial center sums.
      main:  per k-tile [128,3072]: push diff (g - f_orig) on GPSIMD+DVE,
             pull diff (g - cross) on DVE, Square+accumulate over D
             (ACT x4, DVE tensor_tensor_reduce x2) -> d^2 per
             (sample-half, k, part).
      tail:  sqrt, hinge = relu((d_pull + margin - d_push) * w) accumulated
             per partition -> [128,1] partial out.
  - host: sum the 8 partial outputs, divide by 6 * id_count.

Partition layout: q = 2*local_b + h, h = p//3; free dim = (pp = p%3, d).
"""
import re as _re
import sys

import numpy as np

if "/opt/trn_rl_repo" not in sys.path:
    sys.path.insert(0, "/opt/trn_rl_repo")

import bass_rust
import ml_dtypes
import concourse.bass as bass
import concourse.mybir as mybir
import concourse.bass_utils as bass_utils
from concourse import tile
from concourse.vector_clock import ScopedClock

F32 = mybir.dt.float32
BF16 = mybir.dt.bfloat16
AFT = mybir.ActivationFunctionType
BF = ml_dtypes.bfloat16

MARGIN = 0.2
B, K, P, D = 512, 8, 6, 1024
NID = 64
NCORES = 8
BC = B // NCORES          # 64 local samples per core
HFREE = 3 * D             # 3072 free elements per partition row
NQ = 2 * BC               # 128 partition rows
NT = B // 128             # 4 global sample chunks for the centers matmul

# The walrus build in this image rejects instructions carrying more than one
# semaphore wait ("Too many sync wait commands"). Two mitigations:
#  1. TileContext's exit drain normally carries every outstanding wait ->
#     patched to spread waits over a chain of SP nops.
#  2. A post-pass splits any remaining multi-wait instruction by hoisting
#     excess waits onto same-engine nops inserted right before it.
_MAX_WAITS = 1


def _patched_drain_and_barrier(self, tick_clock, wait_clock):
    gc = tick_clock.global_clock
    vals = [int(s) for s in _re.findall(r"-?\d+", repr(gc))]
    procs = [p for p, v in enumerate(vals) if v > 0]
    for i in range(0, len(procs), _MAX_WAITS):
        sub = bass_rust.VectorClock()
        for p in procs[i : i + _MAX_WAITS]:
            sub.require_at_least(p, vals[p])
        nop = self.nc.sync.nop(nofuse=True, hint="drain_wait_split")
        wait_clock.add_sem_waits(nop.ins, ScopedClock({None: sub}))
    self.nc.sync.drain()
    self.nc.all_engine_barrier()
    assert self.sems is not None
    popped = self.nc._tile_sem_poison_stack.pop()
    assert popped is self._sem_poison
    self.nc.clear_and_free_semaphores(list(self.sems.allocated().values()))
    self.nc.all_engine_barrier()


tile.TileContext._drain_and_barrier = _patched_drain_and_barrier


def _split_excess_waits(nc, max_waits=_MAX_WAITS):
    """Hoist excess per-instruction sem waits onto same-engine NoOps."""
    n_split = 0
    for bb in nc.main_func.blocks:
        insts = bb.instructions
        out = []
        for ins in insts:
            si = ins.sync_info
            waits = list(si.on_wait) if si is not None and si.on_wait else []
            if len(waits) > max_waits:
                extra, keep = waits[:-max_waits], waits[-max_waits:]
                for j in range(0, len(extra), max_waits):
                    nop = mybir.InstNoOp(
                        name=f"waitsplit-{n_split}-{j}", ins=[], outs=[]
                    )
                    nop.engine = ins.engine
                    nop.sync_info = mybir.SyncInfo(
                        on_wait=extra[j : j + max_waits], on_update=[]
                    )
                    out.append(nop)
                ins.sync_info = mybir.SyncInfo(
                    on_wait=keep, on_update=list(si.on_update or [])
                )
                n_split += 1
            out.append(ins)
        if len(out) != len(insts):
            bb.instructions = out
    return n_split


_NC_CACHE = None
import os
USE_TTR = os.environ.get("USE_TTR", "1") == "1"


def _build_nc(ntc):
    global _NC_CACHE
    if _NC_CACHE is not None and _NC_CACHE[0] == ntc:
        return _NC_CACHE[1]
    NT = ntc
    nc = bass.Bass()

    fgen = nc.dram_tensor("fgen", [K, NQ, HFREE], BF16, kind="ExternalInput")
    forig = nc.dram_tensor("forig", [NQ, HFREE], BF16, kind="ExternalInput")
    fofull = nc.dram_tensor("fofull", [NT, 128, 2 * HFREE], BF16, kind="ExternalInput")
    mmat = nc.dram_tensor("mmat", [NT, 2, 128, NQ], BF16, kind="ExternalInput")
    wvec = nc.dram_tensor("wvec", [NQ, 1], F32, kind="ExternalInput")
    out = nc.dram_tensor("out", [NQ, 1], F32, kind="ExternalOutput")

    NSEG = 6                # 512-wide matmul chunks per 3072
    SEG = HFREE // NSEG

    with tile.TileContext(nc) as tc:
        with (
            tc.tile_pool(name="const", bufs=1) as cpool,
            tc.tile_pool(name="gpool", bufs=1) as gpool,
            tc.tile_pool(name="dpush", bufs=5) as dpush_pool,
            tc.tile_pool(name="scr", bufs=4) as scr_pool,
            tc.tile_pool(name="psumX", bufs=1, space="PSUM") as psumX,
        ):
            # ---- loads. Small constants via HWDGE (sync). Bulk tiles via
            # SWDGE (gpsimd.dma_start): one SWDGE transfer fans out across
            # all 16 SDMA engines, so tiles arrive in issue order at full
            # aggregate bandwidth instead of trickling at one queue's ~30
            # GB/s. GPSIMD runs no compute in this kernel, so SWDGE
            # descriptor generation has the engine to itself. Order: fo_q
            # and g0 first (gates push k=0), then fof chunks (gate the
            # cross-center matmuls), then the remaining g tiles. ----
            fo_q = cpool.tile([NQ, HFREE], BF16)
            gt = [gpool.tile([NQ, HFREE], BF16, name=f"g{k}", tag=f"g{k}") for k in range(K)]
            fof = [cpool.tile([128, 2 * HFREE], BF16, name=f"fof{t}", tag=f"fof{t}") for t in range(NT)]

            # Two concurrent DMA silos. Each HWDGE queue moves only ~27
            # GB/s, so the latency-critical tiles are striped WIDE across
            # queues (8 partition-slices each for fo_q/g0, 4 for the fof
            # center inputs); the remaining g tiles stream via SWDGE
            # (gpsimd), a separate silo with its own ~160 GB/s aggregate.
            def hload(dst, src, nstripe):
                step = 128 // nstripe
                for s in range(nstripe):
                    sl = slice(s * step, (s + 1) * step)
                    nc.sync.dma_start(dst[sl, :], src[sl, :])

            mm = [
                [cpool.tile([128, NQ], BF16, name=f"mm{t}{h}", tag=f"mm{t}{h}") for h in (0, 1)]
                for t in range(NT)
            ]
            for t in range(NT):
                for h in (0, 1):
                    nc.sync.dma_start(mm[t][h][:], mmat[t, h])
            wv = cpool.tile([NQ, 1], F32, tag="wv")
            nc.sync.dma_start(wv[:], wvec[:])
            nc.gpsimd.dma_start(fo_q[:], forig[:])
            nc.gpsimd.dma_start(gt[0][:], fgen[0])
            for t in range(NT):
                nc.gpsimd.dma_start(fof[t][:], fofull[t])
            for k in range(1, K):
                nc.gpsimd.dma_start(gt[k][:], fgen[k])

            dsq_push = cpool.tile([NQ, 3 * K], F32, tag="dsq_push")
            dsq_pull = cpool.tile([NQ, 3 * K], F32, tag="dsq_pull")

            # Square-accumulates: ACT is the cheap reducer (short
            # accumulator-read); DVE takes ~12 of the 48 segments as fused
            # mult+reduce STTs to balance the two engines.
            def sqacc_act(diff, pp, dst):
                scr = scr_pool.tile([NQ, D], BF16, tag="ascr")
                nc.scalar.activation(
                    scr[:], diff[:, pp * D : (pp + 1) * D], AFT.Square,
                    accum_out=dst,
                )

            def sqacc_dve(diff, pp, dst):
                seg = slice(pp * D, (pp + 1) * D)
                scr = scr_pool.tile([NQ, D], BF16, tag="vscr")
                nc.vector.scalar_tensor_tensor(
                    scr[:], diff[:, seg], 1.0, diff[:, seg],
                    op0=mybir.AluOpType.mult, op1=mybir.AluOpType.mult,
                    accum_out=dst,
                )

            # push[k] depends only on (g_k, fo_q) -> hideable under DMA.
            # All push square-accumulates go to ACT: a DVE STT here could
            # grab the shared SBUF port and starve SWDGE descriptor
            # generation mid-DMA.
            def emit_push(k):
                dpu = dpush_pool.tile([NQ, HFREE], BF16, name=f"dpu{k}", tag="dpu")
                nc.vector.tensor_sub(dpu[:], gt[k][:], fo_q[:])
                col0 = 3 * k
                for pp in range(3):
                    sqacc_act(dpu, pp, dsq_push[:, col0 + pp : col0 + pp + 1])

            def emit_pull(k):
                dpl = dpush_pool.tile([NQ, HFREE], BF16, name=f"dpl{k}", tag="dpl")
                nc.vector.tensor_sub(dpl[:], gt[k][:], cross[:])
                col0 = 3 * k
                for pp in range(3):
                    dst = dsq_pull[:, col0 + pp : col0 + pp + 1]
                    if pp >= 1:
                        sqacc_dve(dpl, pp, dst)
                    else:
                        sqacc_act(dpl, pp, dst)

            for k in range(2):
                emit_push(k)

            # ---- cross centers: (A S)^T @ F, 48 bf16 matmuls, chunk-outer
            # so the PE chain pipelines with the fof chunk arrivals ----
            xps = psumX.tile([NQ, HFREE], F32)
            cross = cpool.tile([NQ, HFREE], BF16, tag="cross")
            for ti, (t, h) in enumerate(
                [(t, h) for t in range(NT) for h in (0, 1)]
            ):
                for j in range(NSEG):
                    seg = slice(j * SEG, (j + 1) * SEG)
                    rhs = fof[t][:, h * HFREE + j * SEG : h * HFREE + (j + 1) * SEG]
                    nc.tensor.matmul(
                        xps[:, seg], mm[t][h][:], rhs,
                        start=(ti == 0), stop=(ti == 2 * NT - 1),
                    )
            nc.scalar.copy(cross[:], xps[:])

            for k in range(2, K):
                emit_push(k)
                emit_pull(k - 2)
            emit_pull(K - 2)
            emit_pull(K - 1)

            # ---- tail ----
            d_push = cpool.tile([NQ, 3 * K], F32, tag="d_push")
            d_pull = cpool.tile([NQ, 3 * K], F32, tag="d_pull")
            nc.scalar.activation(d_push[:], dsq_push[:], AFT.Sqrt)
            nc.scalar.activation(d_pull[:], dsq_pull[:], AFT.Sqrt)
            targ = cpool.tile([NQ, 3 * K], F32, tag="targ")
            # (d_pull + margin) - d_push
            nc.vector.scalar_tensor_tensor(
                targ[:], d_pull[:], MARGIN, d_push[:],
                op0=mybir.AluOpType.add, op1=mybir.AluOpType.subtract,
            )
            relu_scr = cpool.tile([NQ, 3 * K], F32, tag="relu_scr")
            acc = cpool.tile([NQ, 1], F32, tag="acc")
            # relu(w_q * targ) summed over (k, pp); w_q >= 0 so this
            # equals w_q * relu(targ) summed.
            nc.scalar.activation(
                relu_scr[:], targ[:], AFT.Relu, scale=wv[:], accum_out=acc[:]
            )
            nc.sync.dma_start(out[:], acc[:])

    mybir.codegen_inst_isa_subclasses(nc)   # extended-ISA instr bytes (TTR)
    _split_excess_waits(nc)
    _NC_CACHE = (ntc, nc)
    return nc


def _host_prep(f_original, f_generated, pids, camids):
    """Host bookkeeping + per-core input maps."""
    f_original = np.asarray(f_original, dtype=np.float32)
    f_generated = np.asarray(f_generated, dtype=np.float32)
    pids = np.asarray(pids).astype(np.int64)
    camids = np.asarray(camids).astype(np.int64)

    mod = (camids != 0).astype(np.int64)          # 0 = rgb, 1 = sar
    cnt = np.zeros((2, NID), dtype=np.float32)
    np.add.at(cnt, (mod, pids), 1.0)
    valid_id = (cnt[0] > 0) & (cnt[1] > 0)
    id_count = float(valid_id.sum())
    denom = max(id_count, 1.0)

    own_row = (pids + NID * mod).astype(np.int64)          # [B]
    cross_row = (pids + NID * (1 - mod)).astype(np.int64)  # [B]
    inv_cnt2 = (1.0 / np.maximum(cnt.reshape(-1), 1.0)).astype(np.float32)
    grp_cnt = cnt[mod, pids]
    w = np.where(valid_id[pids], 1.0 / (np.maximum(grp_cnt, 1.0) * K), 0.0)
    w = w.astype(np.float32)

    fo_bf = f_original.astype(BF).reshape(B, 2 * HFREE)   # [B, 6144]
    fg_bf = f_generated.astype(BF)

    # Per core, only the f_original rows whose (pid, modality) feeds one of
    # the core's cross centers are needed for the center matmul ("fofc").
    contribs = []
    for c in range(NCORES):
        sl = slice(c * BC, (c + 1) * BC)
        rows_needed = np.unique(cross_row[sl])
        contribs.append(np.nonzero(np.isin(own_row, rows_needed))[0])
    ntc = max(1, max((len(cb) + 127) // 128 for cb in contribs))

    in_maps = []
    for c in range(NCORES):
        sl = slice(c * BC, (c + 1) * BC)
        fg = (
            fg_bf[sl]
            .transpose(1, 0, 2, 3)
            .reshape(K, BC, 2, HFREE)
            .reshape(K, NQ, HFREE)
        )
        fo = fo_bf[sl].reshape(NQ, HFREE)
        contrib = contribs[c]
        npad = ntc * 128
        cpad = np.zeros(npad, dtype=np.int64)
        cpad[: len(contrib)] = contrib
        fofc = fo_bf[cpad].reshape(ntc, 128, 2 * HFREE)
        pos_of = {int(gi): pos for pos, gi in enumerate(contrib)}
        # mmat[t, h, i, q] = inv_cnt[cross_row[b]] where packed sample
        # 128t+i has own_row == cross_row[b(q)] and q = 2b+h.
        mm = np.zeros((ntc, 2, 128, NQ), dtype=np.float32)
        for bl in range(BC):
            b = c * BC + bl
            r = cross_row[b]
            s = inv_cnt2[r]
            for gi in np.nonzero(own_row == r)[0]:
                t, i = divmod(pos_of[int(gi)], 128)
                mm[t, 0, i, 2 * bl] = s
                mm[t, 1, i, 2 * bl + 1] = s
        wq = np.zeros((NQ, 1), dtype=np.float32)
        wq[0::2, 0] = w[sl]
        wq[1::2, 0] = w[sl]
        in_maps.append(
            {
                "fgen": np.ascontiguousarray(fg),
                "forig": np.ascontiguousarray(fo),
                "fofull": np.ascontiguousarray(fofc),
                "mmat": mm.astype(BF),
                "wvec": wq,
            }
        )
    return in_maps, id_count, denom, ntc


def run_device(f_original, f_generated, pids, camids, **spmd_kwargs):
    """Build + run; returns (loss, BassKernelResults)."""
    in_maps, id_count, denom, ntc = _host_prep(f_original, f_generated, pids, camids)
    nc = _build_nc(ntc)
    res = bass_utils.run_bass_kernel_spmd(
        nc, in_maps, core_ids=list(range(NCORES)), **spmd_kwargs
    )
    total = float(sum(r["out"].sum() for r in res.results))
    loss = np.float32(total / (P * denom)) if id_count > 0 else np.float32(0.0)
    return np.asarray(loss, dtype=np.float32), res


def kernel(f_original, f_generated, pids, camids):
    loss, _ = run_device(f_original, f_generated, pids, camids)
    return loss
